# revision 11
# baseline (speedup 1.0000x reference)
"""Multi-head causal attention on 8 TRN2 NeuronCores.

Problem: B=4, S=2048, D=768, H=12 heads (dk=64), causal mask, f32.

Sharding: 8 cores = 4 batches x 2 head-groups (6 heads each).
Core c handles batch c//2 and heads [6*(c%2), 6*(c%2)+6).
Each core computes its partial output projection (over its 384 local
features) in bf16; the pair-sum (tensor-parallel all-reduce after Wo)
and the bo bias add happen at unshard time on the host in f32.

Design (v2), engine-balance driven:
 - ACT (ScalarE) exp is the hard floor: 1 elem/lane/cycle. The exp of
   both heads of a pair is issued as ONE instruction over a [128,2,W]
   PSUM view (adjacent quad slots) to halve the 352-cycle/instr
   overhead.
 - Scores matmuls are K=64 and run 2x via PE row tiling: head A on
   tile (0,0) (kt/qt partitions 0-63), head B on (64,0). Verified on
   HW: paired K=64 N=512 matmuls take 132ns vs 261ns serial. The A/B
   slot pair is freed by a single exp, so no pool-WAR semaphore lands
   between the paired matmuls.
 - Attention i-block width W=512. PSUM: quad [128,4,512] (4 banks,
   double-buffered slot pairs) + pctx A/B (2 banks) + aux [128,2,512]
   (2 banks, shared by projection fillers / out-projection).
 - All projection work (v, q/k of later pairs, out-projection of done
   i-blocks) is interleaved into the attention stream as PE filler so
   the PE never idles (keeps the HAM clock gate at full speed) while
   ACT streams exps.
 - Softmax denominator rides the ctx matmul as a ones-column (M=65);
   epilogue: gpsimd partition-broadcast of the denom row + one DVE
   divide straight into ctxt (bf16).
"""

import os
import numpy as np
import ml_dtypes

import concourse.bass as bass
import concourse.tile as tile
import concourse.mybir as mybir
from concourse import bacc
from concourse.masks import make_identity

B, S, D, H = 4, 2048, 768, 12
DK, P = 64, 128
HL = H // 2            # 6 heads per core
DL = HL * DK           # 384 local features
KD = D // P            # 6 contraction chunks over d
MT = DL // P           # 3 head-pairs (row-tiles of qT/kT/ctxT)
ST = S // P            # 16 s-tiles
W = 512                # attention i-block width
NIB = S // W           # 4 i-blocks
NEG = -1e9

CDT = mybir.dt.bfloat16
NP_CDT = ml_dtypes.bfloat16
F32 = mybir.dt.float32

N_CORES = 8


def _emit(nc, tc, xt_d, wq_d, wk_d, wv_d, wo_d, out_d):
    Exp = mybir.ActivationFunctionType.Exp
    Div = mybir.AluOpType.divide

    with tc.tile_pool(name="persist", bufs=1) as per, \
         tc.tile_pool(name="quad", bufs=1, space="PSUM") as qp, \
         tc.tile_pool(name="pc", bufs=2, space="PSUM") as cp, \
         tc.tile_pool(name="aux", bufs=1, space="PSUM") as ap, \
         tc.tile_pool(name="sb_e", bufs=2) as ep, \
         tc.tile_pool(name="sb_l", bufs=2) as lp, \
         tc.tile_pool(name="sb_o", bufs=2) as ob:
        xt = per.tile([P, KD, S], CDT)
        wq = per.tile([P, KD, DL], CDT)
        wk = per.tile([P, KD, DL], CDT)
        wv = per.tile([P, KD, DL], CDT)
        wo = per.tile([P, MT, D], CDT)
        qt = per.tile([P, MT, S], CDT)
        kt = per.tile([P, MT, S], CDT)
        v = per.tile([P, ST, HL, DK + 2], CDT)  # [v | ones | pad]
        ctxt = per.tile([P, MT, S], CDT)
        ident = per.tile([P, P], CDT)
        maskm = per.tile([P, P], CDT)

        # aux PSUM region: projection segments / out-projection tiles.
        aux = ap.tile([P, 2, 512], F32)

        # DMA priority: q/k path first (phase-0 projections), then wv,
        # wo last (needed only by the out-projection fillers).
        for k in range(KD):
            nc.sync.dma_start(out=xt[:, k, :], in_=xt_d[k * P:(k + 1) * P, :])
            nc.sync.dma_start(out=wq[:, k, :], in_=wq_d[k * P:(k + 1) * P, :])
            nc.sync.dma_start(out=wk[:, k, :], in_=wk_d[k * P:(k + 1) * P, :])
        for k in range(KD):
            nc.sync.dma_start(out=wv[:, k, :], in_=wv_d[k * P:(k + 1) * P, :])
        for m in range(MT):
            nc.sync.dma_start(out=wo[:, m, :], in_=wo_d[m * P:(m + 1) * P, :])

        nc.vector.memset(v[:, :, :, DK:], 0.0)
        nc.vector.memset(v[:, :, :, DK], 1.0)
        # Causal mask for diagonal tiles is injected on the PE itself:
        # matmul(lhsT=ident, rhs=maskm) writes maskm into the PSUM region
        # as the accumulation-group opener; the k.q matmul accumulates on
        # top. maskm[j, i] = NEG where i < j (invalid), 0 where i >= j.
        make_identity(nc, ident)
        nc.gpsimd.memset(maskm, 0.0)
        nc.gpsimd.affine_select(
            out=maskm, in_=maskm, compare_op=mybir.AluOpType.is_ge,
            fill=NEG, base=0, pattern=[[1, P]], channel_multiplier=-1)

        # HAM warmup: dummy matmuls keep the PE active window busy while
        # the input DMAs land. Output never read.
        for _ in range(40):
            nc.tensor.matmul(aux[:, 0, 0:P], lhsT=ident, rhs=maskm,
                             start=True, stop=True)

        # ---- projection emitters (also used as PE filler) ----
        def emit_qk_seg(wt, dst, mh, seg):
            slot = seg % 2
            s0 = seg * 512
            for k in range(KD):
                nc.tensor.matmul(
                    aux[:, slot, :], lhsT=wt[:, k, mh * P:(mh + 1) * P],
                    rhs=xt[:, k, s0:s0 + 512],
                    start=(k == 0), stop=(k == KD - 1))
            nc.vector.tensor_copy(dst[:, mh, s0:s0 + 512], aux[:, slot, :])

        def emit_v_st(st):
            slot = st % 2
            for k in range(KD):
                nc.tensor.matmul(
                    aux[:, slot, 0:DL], lhsT=xt[:, k, st * P:(st + 1) * P],
                    rhs=wv[:, k, :], start=(k == 0), stop=(k == KD - 1))
            nc.vector.tensor_copy(v[:, st, :, 0:DK], aux[:, slot, 0:DL])

        def emit_outproj_st(st):
            for n0, nn, slot in ((0, 512, 0), (512, 256, 1)):
                for m in range(MT):
                    nc.tensor.matmul(
                        aux[:, slot, 0:nn],
                        lhsT=ctxt[:, m, st * P:(st + 1) * P],
                        rhs=wo[:, m, n0:n0 + nn],
                        start=(m == 0), stop=(m == MT - 1))
            osb = ob.tile([P, D], CDT, tag="osb", name=f"osb_{st}")
            nc.vector.tensor_copy(osb[:, 0:512], aux[:, 0, :])
            nc.vector.tensor_copy(osb[:, 512:D], aux[:, 1, 0:D - 512])
            nc.sync.dma_start(out=out_d[st * P:(st + 1) * P, :], in_=osb)

        # ---- filler queue: (deadline t = mh*NIB+ib, closure) ----
        fillers = []

        def fill(n=1):
            for _ in range(n):
                if fillers:
                    fillers.pop(0)[1]()

        def flush(t):
            while fillers and fillers[0][0] <= t:
                fillers.pop(0)[1]()

        # ---- attention for one pair, one i-block ----
        def emit_att(mh, ib):
            heads = (2 * mh, 2 * mh + 1)
            i0 = ib * W
            njt = (i0 + W) // P
            quad = qp.tile([P, 4, W], F32, tag="quad", name=f"quad_{mh}_{ib}")
            pctxs = {h: cp.tile([P, W], F32, tag="pc",
                                name=f"pctx_{h}_{ib}") for h in heads}
            for jt in range(njt):
                sp = 2 * (jt % 2)
                c0 = max(0, jt * P - i0)
                diag = jt * P >= i0
                for idx, h in enumerate(heads):
                    oh = idx * DK
                    slot = quad[:, sp + idx, :]
                    ktt = kt[oh:oh + DK, mh, jt * P:(jt + 1) * P]
                    if diag:
                        nc.tensor.matmul(slot[:, c0:c0 + P], lhsT=ident,
                                         rhs=maskm, start=True, stop=False)
                        nc.tensor.matmul(
                            slot[:, c0:c0 + P], lhsT=ktt,
                            rhs=qt[oh:oh + DK, mh, i0 + c0:i0 + c0 + P],
                            start=False, stop=True)
                        if c0 + P < W:
                            nc.tensor.matmul(
                                slot[:, c0 + P:W], lhsT=ktt,
                                rhs=qt[oh:oh + DK, mh, i0 + c0 + P:i0 + W],
                                start=True, stop=True)
                    else:
                        nc.tensor.matmul(
                            slot, lhsT=ktt,
                            rhs=qt[oh:oh + DK, mh, i0:i0 + W],
                            start=True, stop=True)
                et = ep.tile([P, 2, W], CDT, tag="et",
                             name=f"et_{mh}_{ib}_{jt}")
                nc.scalar.activation(et[:, :, c0:W], quad[:, sp:sp + 2, c0:W],
                                     Exp, scale=0.125)
                for idx, h in enumerate(heads):
                    nc.tensor.matmul(
                        pctxs[h][0:DK + 2, c0:W],
                        lhsT=v[:, jt, h, :], rhs=et[:, idx, c0:W],
                        start=(jt == 0), stop=(jt == njt - 1))
                if jt % 2 == 1:
                    fill(1)
            # epilogue: one fast copy evacuates ctx rows + denom row to
            # SBUF (frees the pctx PSUM bank for the next i-block); the
            # normalize (recip+broadcast+mul) then runs off SBUF, fully
            # overlapped with the next block's attention.
            for idx, h in enumerate(heads):
                ev = lp.tile([DK, W], F32, tag="ev", name=f"ev_{h}_{ib}")
                nc.vector.tensor_copy(ev, pctxs[h][0:DK, :])
                lsb = lp.tile([1, W], F32, tag="lsb", name=f"lsb_{h}_{ib}")
                nc.vector.tensor_copy(lsb, pctxs[h][DK:DK + 1, :])
                rsb = lp.tile([1, W], F32, tag="rsb", name=f"rsb_{h}_{ib}")
                nc.vector.reciprocal_approx_fast(rsb, lsb)
                lb = lp.tile([DK, W], F32, tag="lb", name=f"lb_{h}_{ib}")
                nc.gpsimd.partition_broadcast(lb, rsb, channels=DK)
                nc.vector.tensor_mul(
                    ctxt[idx * DK:(idx + 1) * DK, mh, i0:i0 + W],
                    ev, lb)

        # ---- phase 0: minimum to start (0, ib0): q/k seg0 + v st0..3 ----
        emit_qk_seg(wq, qt, 0, 0)
        emit_qk_seg(wk, kt, 0, 0)
        for st in range(4):
            emit_v_st(st)

        # ---- fillers with deadlines (t = mh*NIB + ib) ----
        # qk(p, seg) needed by (p, ib=seg); v(st) by (0, ib=st//4);
        # out-projections are appended during pair 2.
        for p in range(MT):
            for seg in range(4):
                if p == 0 and seg == 0:
                    continue
                fillers.append((p * NIB + seg,
                                lambda p=p, seg=seg: emit_qk_seg(wq, qt, p, seg)))
                fillers.append((p * NIB + seg,
                                lambda p=p, seg=seg: emit_qk_seg(wk, kt, p, seg)))
        for st in range(4, ST):
            fillers.append((st // 4, lambda st=st: emit_v_st(st)))
        fillers.sort(key=lambda x: x[0])

        # ---- main loop: pair-outer, i-block inner ----
        for mh in range(MT):
            for ib in range(NIB):
                flush(mh * NIB + ib)
                emit_att(mh, ib)
                if mh == MT - 1:
                    # this i-block's ctxt is now complete for all pairs:
                    # queue its out-projection as filler for later blocks
                    for st in range(ib * (W // P), (ib + 1) * (W // P)):
                        fillers.append((99, lambda st=st: emit_outproj_st(st)))
        while fillers:
            fill(1)


def build_nc():
    nc = bacc.Bacc(trn_type="TRN2", target_bir_lowering=False, debug=False)
    xt_d = nc.dram_tensor("xt", [D, S], CDT, kind="ExternalInput").ap()
    wq_d = nc.dram_tensor("wq", [D, DL], CDT, kind="ExternalInput").ap()
    wk_d = nc.dram_tensor("wk", [D, DL], CDT, kind="ExternalInput").ap()
    wv_d = nc.dram_tensor("wv", [D, DL], CDT, kind="ExternalInput").ap()
    wo_d = nc.dram_tensor("wo", [DL, D], CDT, kind="ExternalInput").ap()
    out_d = nc.dram_tensor("out", [S, D], CDT, kind="ExternalOutput").ap()
    with tile.TileContext(nc) as tc:
        _emit(nc, tc, xt_d, wq_d, wk_d, wv_d, wo_d, out_d)
    nc.compile()
    return nc


def make_in_maps(x, Wq, Wk, Wv, Wo):
    in_maps = []
    for c in range(N_CORES):
        b, g = c // 2, c % 2
        hsl = slice(g * DL, (g + 1) * DL)
        in_maps.append({
            "xt": np.ascontiguousarray(x[b].T).astype(NP_CDT),
            "wq": np.ascontiguousarray(Wq[hsl, :].T).astype(NP_CDT),
            "wk": np.ascontiguousarray(Wk[hsl, :].T).astype(NP_CDT),
            "wv": np.ascontiguousarray(Wv[hsl, :].T).astype(NP_CDT),
            "wo": np.ascontiguousarray(Wo[:, hsl].T).astype(NP_CDT),
        })
    return in_maps


_BUILT = None
LAST_RESULT = None


def _install_ntff_hook():
    """Provide the antenv.axon_hooks module run_bass_kernel_spmd expects
    for NTFF profiling under axon (the agent image ships only a stub
    antenv package)."""
    import sys
    import types
    if "antenv.axon_hooks" in sys.modules:
        return
    mod = types.ModuleType("antenv.axon_hooks")
    mod._hook = None

    def set_axon_ntff_profile_hook(h):
        mod._hook = h

    def get_axon_ntff_profile_hook():
        return mod._hook

    mod.set_axon_ntff_profile_hook = set_axon_ntff_profile_hook
    mod.get_axon_ntff_profile_hook = get_axon_ntff_profile_hook
    sys.modules["antenv.axon_hooks"] = mod
    import antenv
    antenv.axon_hooks = mod
    try:
        from trn_agent_boot.trn_boot import _ntff_profile_via_ctypes
        hook = _ntff_profile_via_ctypes("/opt/axon/libaxon_pjrt.so")
        if hook is not None:
            mod._hook = hook
    except Exception:
        pass


def kernel(**inputs):
    global _BUILT, LAST_RESULT
    from concourse.bass_utils import run_bass_kernel_spmd

    x = np.asarray(inputs["x"], np.float32)
    Wq = np.asarray(inputs["Wq"], np.float32)
    Wk = np.asarray(inputs["Wk"], np.float32)
    Wv = np.asarray(inputs["Wv"], np.float32)
    Wo = np.asarray(inputs["Wo"], np.float32)
    bo = np.asarray(inputs["bo"], np.float32)

    if _BUILT is None:
        _BUILT = build_nc()
    nc = _BUILT

    trace = bool(int(os.environ.get("KTRACE", "0")))
    if trace:
        _install_ntff_hook()
    in_maps = make_in_maps(x, Wq, Wk, Wv, Wo)
    res = run_bass_kernel_spmd(
        nc, in_maps, core_ids=list(range(N_CORES)), trace=trace)
    LAST_RESULT = res

    out = np.empty((B, S, D), np.float32)
    for b in range(B):
        out[b] = (res.results[2 * b]["out"].astype(np.float32)
                  + res.results[2 * b + 1]["out"].astype(np.float32))
    out += bo
    return out


# revision 12
# speedup vs baseline: 1.3345x; 1.3345x over previous
"""Multi-head causal attention on 8 TRN2 NeuronCores.

Problem: B=4, S=2048, D=768, H=12 heads (dk=64), causal mask, f32.

Sharding: 8 cores = 4 batches x 2 head-groups (6 heads each).
Core c handles batch c//2 and heads [6*(c%2), 6*(c%2)+6).
Each core computes its partial output projection (over its 384 local
features) in bf16; the pair-sum (tensor-parallel all-reduce after Wo)
and the bo bias add happen at unshard time on the host in f32.

Design (v2), engine-balance driven:
 - ACT (ScalarE) exp is the hard floor: 1 elem/lane/cycle. The exp of
   both heads of a pair is issued as ONE instruction over a [128,2,W]
   PSUM view (adjacent quad slots) to halve the 352-cycle/instr
   overhead.
 - Scores matmuls are K=64 and run 2x via PE row tiling: head A on
   tile (0,0) (kt/qt partitions 0-63), head B on (64,0). Verified on
   HW: paired K=64 N=512 matmuls take 132ns vs 261ns serial. The A/B
   slot pair is freed by a single exp, so no pool-WAR semaphore lands
   between the paired matmuls.
 - Attention i-block width W=512. PSUM: quad [128,4,512] (4 banks,
   double-buffered slot pairs) + pctx A/B (2 banks) + aux [128,2,512]
   (2 banks, shared by projection fillers / out-projection).
 - All projection work (v, q/k of later pairs, out-projection of done
   i-blocks) is interleaved into the attention stream as PE filler so
   the PE never idles (keeps the HAM clock gate at full speed) while
   ACT streams exps.
 - Softmax denominator rides the ctx matmul as a ones-column (M=65);
   epilogue: gpsimd partition-broadcast of the denom row + one DVE
   divide straight into ctxt (bf16).
"""

import os
import numpy as np
import ml_dtypes

import concourse.bass as bass
import concourse.tile as tile
import concourse.mybir as mybir
from concourse import bacc
from concourse.masks import make_identity

B, S, D, H = 4, 2048, 768, 12
DK, P = 64, 128
HL = H // 2            # 6 heads per core
DL = HL * DK           # 384 local features
KD = D // P            # 6 contraction chunks over d
MT = DL // P           # 3 head-pairs (row-tiles of qT/kT/ctxT)
ST = S // P            # 16 s-tiles
W = 512                # attention i-block width
NIB = S // W           # 4 i-blocks
NEG = -1e9

CDT = mybir.dt.bfloat16
NP_CDT = ml_dtypes.bfloat16
F32 = mybir.dt.float32

N_CORES = 8


def _emit(nc, tc, xt_d, wq_d, wk_d, wv_d, wo_d, out_d):
    Exp = mybir.ActivationFunctionType.Exp
    Div = mybir.AluOpType.divide

    with tc.tile_pool(name="persist", bufs=1) as per, \
         tc.tile_pool(name="quad", bufs=1, space="PSUM") as qp, \
         tc.tile_pool(name="pc", bufs=2, space="PSUM") as cp, \
         tc.tile_pool(name="aux", bufs=1, space="PSUM") as ap, \
         tc.tile_pool(name="sb_e", bufs=2) as ep, \
         tc.tile_pool(name="sb_l", bufs=2) as lp, \
         tc.tile_pool(name="sb_o", bufs=2) as ob:
        xt = per.tile([P, KD, S], CDT)
        wq = per.tile([P, KD, DL], CDT)
        wk = per.tile([P, KD, DL], CDT)
        wv = per.tile([P, KD, DL], CDT)
        wo = per.tile([P, MT, D], CDT)
        qt = per.tile([P, MT, S], CDT)
        kt = per.tile([P, MT, S], CDT)
        v = per.tile([P, ST, HL, DK + 2], CDT)  # [v | ones | pad]
        ctxt = per.tile([P, MT, S], CDT)
        ident = per.tile([P, P], CDT)
        maskm = per.tile([P, P], CDT)

        # aux PSUM region: projection segments / out-projection tiles.
        aux = ap.tile([P, 2, 512], F32)

        # DMA priority: q/k path first (phase-0 projections), then wv,
        # wo last (needed only by the out-projection fillers).
        HS = S // 2
        for k in range(KD):
            nc.sync.dma_start(out=xt[:, k, 0:HS],
                              in_=xt_d[k * P:(k + 1) * P, 0:HS])
        for k in range(KD):
            nc.sync.dma_start(out=wq[:, k, :], in_=wq_d[k * P:(k + 1) * P, :])
            nc.sync.dma_start(out=wk[:, k, :], in_=wk_d[k * P:(k + 1) * P, :])
        for k in range(KD):
            nc.sync.dma_start(out=wv[:, k, :], in_=wv_d[k * P:(k + 1) * P, :])
        for k in range(KD):
            nc.sync.dma_start(out=xt[:, k, HS:S],
                              in_=xt_d[k * P:(k + 1) * P, HS:S])
        for m in range(MT):
            nc.sync.dma_start(out=wo[:, m, :], in_=wo_d[m * P:(m + 1) * P, :])

        nc.vector.memset(v[:, :, :, DK:], 0.0)
        nc.vector.memset(v[:, :, :, DK], 1.0)
        # Causal mask for diagonal tiles is injected on the PE itself:
        # matmul(lhsT=ident, rhs=maskm) writes maskm into the PSUM region
        # as the accumulation-group opener; the k.q matmul accumulates on
        # top. maskm[j, i] = NEG where i < j (invalid), 0 where i >= j.
        make_identity(nc, ident)
        nc.gpsimd.memset(maskm, 0.0)
        nc.gpsimd.affine_select(
            out=maskm, in_=maskm, compare_op=mybir.AluOpType.is_ge,
            fill=NEG, base=0, pattern=[[1, P]], channel_multiplier=-1)

        # HAM warmup: dummy matmuls keep the PE active window busy while
        # the input DMAs land. Output never read.
        for _ in range(40):
            nc.tensor.matmul(aux[:, 0, 0:P], lhsT=ident, rhs=maskm,
                             start=True, stop=True)

        # ---- projection emitters (also used as PE filler) ----
        def emit_qk_seg(wt, dst, mh, seg):
            slot = seg % 2
            s0 = seg * 512
            for k in range(KD):
                nc.tensor.matmul(
                    aux[:, slot, :], lhsT=wt[:, k, mh * P:(mh + 1) * P],
                    rhs=xt[:, k, s0:s0 + 512],
                    start=(k == 0), stop=(k == KD - 1))
            nc.vector.tensor_copy(dst[:, mh, s0:s0 + 512], aux[:, slot, :])

        def emit_v_st(st):
            slot = st % 2
            for k in range(KD):
                nc.tensor.matmul(
                    aux[:, slot, 0:DL], lhsT=xt[:, k, st * P:(st + 1) * P],
                    rhs=wv[:, k, :], start=(k == 0), stop=(k == KD - 1))
            nc.vector.tensor_copy(v[:, st, :, 0:DK], aux[:, slot, 0:DL])

        def emit_outproj_st(st):
            for n0, nn, slot in ((0, 512, 0), (512, 256, 1)):
                for m in range(MT):
                    nc.tensor.matmul(
                        aux[:, slot, 0:nn],
                        lhsT=ctxt[:, m, st * P:(st + 1) * P],
                        rhs=wo[:, m, n0:n0 + nn],
                        start=(m == 0), stop=(m == MT - 1))
            osb = ob.tile([P, D], CDT, tag="osb", name=f"osb_{st}")
            nc.vector.tensor_copy(osb[:, 0:512], aux[:, 0, :])
            nc.vector.tensor_copy(osb[:, 512:D], aux[:, 1, 0:D - 512])
            nc.sync.dma_start(out=out_d[st * P:(st + 1) * P, :], in_=osb)

        # ---- filler queue: (deadline t = mh*NIB+ib, closure) ----
        fillers = []

        def fill(n=1):
            for _ in range(n):
                if fillers:
                    fillers.pop(0)[1]()

        def flush(t):
            while fillers and fillers[0][0] <= t:
                fillers.pop(0)[1]()

        # ---- attention for one pair, one i-block ----
        def emit_att(mh, ib):
            heads = (2 * mh, 2 * mh + 1)
            i0 = ib * W
            njt = (i0 + W) // P
            quad = qp.tile([P, 4, W], F32, tag="quad", name=f"quad_{mh}_{ib}")
            pctxs = {h: cp.tile([P, W], F32, tag="pc",
                                name=f"pctx_{h}_{ib}") for h in heads}
            ets = {}
            for jt in range(njt + 1):
                # stage 1: scores + exp for jt
                if jt < njt:
                    sp = 2 * (jt % 2)
                    c0 = max(0, jt * P - i0)
                    diag = jt * P >= i0
                    for idx, h in enumerate(heads):
                        oh = idx * DK
                        slot = quad[:, sp + idx, :]
                        ktt = kt[oh:oh + DK, mh, jt * P:(jt + 1) * P]
                        if diag:
                            nc.tensor.matmul(slot[:, c0:c0 + P], lhsT=ident,
                                             rhs=maskm, start=True, stop=False)
                            nc.tensor.matmul(
                                slot[:, c0:c0 + P], lhsT=ktt,
                                rhs=qt[oh:oh + DK, mh, i0 + c0:i0 + c0 + P],
                                start=False, stop=True)
                            if c0 + P < W:
                                nc.tensor.matmul(
                                    slot[:, c0 + P:W], lhsT=ktt,
                                    rhs=qt[oh:oh + DK, mh, i0 + c0 + P:i0 + W],
                                    start=True, stop=True)
                        else:
                            nc.tensor.matmul(
                                slot, lhsT=ktt,
                                rhs=qt[oh:oh + DK, mh, i0:i0 + W],
                                start=True, stop=True)
                    et = ep.tile([P, 2, W], CDT, tag="et",
                                 name=f"et_{mh}_{ib}_{jt}")
                    nc.scalar.activation(et[:, :, c0:W],
                                         quad[:, sp:sp + 2, c0:W],
                                         Exp, scale=0.125)
                    ets[jt] = et
                # stage 2 (one jt behind): ctx for jt-1 -- by emission
                # priority the PE gets scores(jt) between exp(jt-1) and
                # ctx(jt-1), so the frozen engine order never blocks on ACT
                if jt >= 1:
                    pj = jt - 1
                    pc0 = max(0, pj * P - i0)
                    for idx, h in enumerate(heads):
                        nc.tensor.matmul(
                            pctxs[h][0:DK + 2, pc0:W],
                            lhsT=v[:, pj, h, :], rhs=ets[pj][:, idx, pc0:W],
                            start=(pj == 0), stop=(pj == njt - 1))
                    del ets[pj]
                fill(1)
            # epilogue: one fast copy evacuates ctx rows + denom row to
            # SBUF (frees the pctx PSUM bank for the next i-block); the
            # normalize (recip+broadcast+mul) then runs off SBUF, fully
            # overlapped with the next block's attention.
            for idx, h in enumerate(heads):
                ev = lp.tile([DK, W], F32, tag="ev", name=f"ev_{h}_{ib}")
                nc.vector.tensor_copy(ev, pctxs[h][0:DK, :])
                lsb = lp.tile([1, W], F32, tag="lsb", name=f"lsb_{h}_{ib}")
                nc.vector.tensor_copy(lsb, pctxs[h][DK:DK + 1, :])
                rsb = lp.tile([1, W], F32, tag="rsb", name=f"rsb_{h}_{ib}")
                nc.vector.reciprocal_approx_fast(rsb, lsb)
                lb = lp.tile([DK, W], F32, tag="lb", name=f"lb_{h}_{ib}")
                nc.gpsimd.partition_broadcast(lb, rsb, channels=DK)
                nc.vector.tensor_mul(
                    ctxt[idx * DK:(idx + 1) * DK, mh, i0:i0 + W],
                    ev, lb)

        # ---- phase 0: minimum to start (0, ib0): q/k seg0 + v st0..3 ----
        emit_qk_seg(wq, qt, 0, 0)
        emit_qk_seg(wk, kt, 0, 0)
        for st in range(4):
            emit_v_st(st)

        # ---- fillers with deadlines (t = mh*NIB + ib) ----
        # qk(p, seg) needed by (p, ib=seg); v(st) by (0, ib=st//4);
        # out-projections are appended during pair 2.
        for p in range(MT):
            for seg in range(4):
                if p == 0 and seg == 0:
                    continue
                fillers.append((p * NIB + seg,
                                lambda p=p, seg=seg: emit_qk_seg(wq, qt, p, seg)))
                fillers.append((p * NIB + seg,
                                lambda p=p, seg=seg: emit_qk_seg(wk, kt, p, seg)))
        for st in range(4, ST):
            fillers.append((st // 4, lambda st=st: emit_v_st(st)))
        fillers.sort(key=lambda x: x[0])

        # ---- main loop: pair-outer, i-block inner ----
        for mh in range(MT):
            for ib in range(NIB):
                flush(mh * NIB + ib)
                emit_att(mh, ib)
                if mh == MT - 1:
                    # this i-block's ctxt is now complete for all pairs:
                    # queue its out-projection as filler for later blocks
                    for st in range(ib * (W // P), (ib + 1) * (W // P)):
                        fillers.append((99, lambda st=st: emit_outproj_st(st)))
        while fillers:
            fill(1)


def build_nc():
    nc = bacc.Bacc(trn_type="TRN2", target_bir_lowering=False, debug=False)
    xt_d = nc.dram_tensor("xt", [D, S], CDT, kind="ExternalInput").ap()
    wq_d = nc.dram_tensor("wq", [D, DL], CDT, kind="ExternalInput").ap()
    wk_d = nc.dram_tensor("wk", [D, DL], CDT, kind="ExternalInput").ap()
    wv_d = nc.dram_tensor("wv", [D, DL], CDT, kind="ExternalInput").ap()
    wo_d = nc.dram_tensor("wo", [DL, D], CDT, kind="ExternalInput").ap()
    out_d = nc.dram_tensor("out", [S, D], CDT, kind="ExternalOutput").ap()
    with tile.TileContext(nc) as tc:
        _emit(nc, tc, xt_d, wq_d, wk_d, wv_d, wo_d, out_d)
    nc.compile()
    return nc


def make_in_maps(x, Wq, Wk, Wv, Wo):
    in_maps = []
    for c in range(N_CORES):
        b, g = c // 2, c % 2
        hsl = slice(g * DL, (g + 1) * DL)
        in_maps.append({
            "xt": np.ascontiguousarray(x[b].T).astype(NP_CDT),
            "wq": np.ascontiguousarray(Wq[hsl, :].T).astype(NP_CDT),
            "wk": np.ascontiguousarray(Wk[hsl, :].T).astype(NP_CDT),
            "wv": np.ascontiguousarray(Wv[hsl, :].T).astype(NP_CDT),
            "wo": np.ascontiguousarray(Wo[:, hsl].T).astype(NP_CDT),
        })
    return in_maps


_BUILT = None
LAST_RESULT = None


def _install_ntff_hook():
    """Provide the antenv.axon_hooks module run_bass_kernel_spmd expects
    for NTFF profiling under axon (the agent image ships only a stub
    antenv package)."""
    import sys
    import types
    if "antenv.axon_hooks" in sys.modules:
        return
    mod = types.ModuleType("antenv.axon_hooks")
    mod._hook = None

    def set_axon_ntff_profile_hook(h):
        mod._hook = h

    def get_axon_ntff_profile_hook():
        return mod._hook

    mod.set_axon_ntff_profile_hook = set_axon_ntff_profile_hook
    mod.get_axon_ntff_profile_hook = get_axon_ntff_profile_hook
    sys.modules["antenv.axon_hooks"] = mod
    import antenv
    antenv.axon_hooks = mod
    try:
        from trn_agent_boot.trn_boot import _ntff_profile_via_ctypes
        hook = _ntff_profile_via_ctypes("/opt/axon/libaxon_pjrt.so")
        if hook is not None:
            mod._hook = hook
    except Exception:
        pass


def kernel(**inputs):
    global _BUILT, LAST_RESULT
    from concourse.bass_utils import run_bass_kernel_spmd

    x = np.asarray(inputs["x"], np.float32)
    Wq = np.asarray(inputs["Wq"], np.float32)
    Wk = np.asarray(inputs["Wk"], np.float32)
    Wv = np.asarray(inputs["Wv"], np.float32)
    Wo = np.asarray(inputs["Wo"], np.float32)
    bo = np.asarray(inputs["bo"], np.float32)

    if _BUILT is None:
        _BUILT = build_nc()
    nc = _BUILT

    trace = bool(int(os.environ.get("KTRACE", "0")))
    if trace:
        _install_ntff_hook()
    in_maps = make_in_maps(x, Wq, Wk, Wv, Wo)
    res = run_bass_kernel_spmd(
        nc, in_maps, core_ids=list(range(N_CORES)), trace=trace)
    LAST_RESULT = res

    out = np.empty((B, S, D), np.float32)
    for b in range(B):
        out[b] = (res.results[2 * b]["out"].astype(np.float32)
                  + res.results[2 * b + 1]["out"].astype(np.float32))
    out += bo
    return out


# revision 13
# speedup vs baseline: 1.3486x; 1.0106x over previous
"""Multi-head causal attention on 8 TRN2 NeuronCores.

Problem: B=4, S=2048, D=768, H=12 heads (dk=64), causal mask, f32.

Sharding: 8 cores = 4 batches x 2 head-groups (6 heads each).
Core c handles batch c//2 and heads [6*(c%2), 6*(c%2)+6).
Each core computes its partial output projection (over its 384 local
features) in bf16; the pair-sum (tensor-parallel all-reduce after Wo)
and the bo bias add happen at unshard time on the host in f32.

Design (v2), engine-balance driven:
 - ACT (ScalarE) exp is the hard floor: 1 elem/lane/cycle. The exp of
   both heads of a pair is issued as ONE instruction over a [128,2,W]
   PSUM view (adjacent quad slots) to halve the 352-cycle/instr
   overhead.
 - Scores matmuls are K=64 and run 2x via PE row tiling: head A on
   tile (0,0) (kt/qt partitions 0-63), head B on (64,0). Verified on
   HW: paired K=64 N=512 matmuls take 132ns vs 261ns serial. The A/B
   slot pair is freed by a single exp, so no pool-WAR semaphore lands
   between the paired matmuls.
 - Attention i-block width W=512. PSUM: quad [128,4,512] (4 banks,
   double-buffered slot pairs) + pctx A/B (2 banks) + aux [128,2,512]
   (2 banks, shared by projection fillers / out-projection).
 - All projection work (v, q/k of later pairs, out-projection of done
   i-blocks) is interleaved into the attention stream as PE filler so
   the PE never idles (keeps the HAM clock gate at full speed) while
   ACT streams exps.
 - Softmax denominator rides the ctx matmul as a ones-column (M=65);
   epilogue: gpsimd partition-broadcast of the denom row + one DVE
   divide straight into ctxt (bf16).
"""

import os
import numpy as np
import ml_dtypes

import concourse.bass as bass
import concourse.tile as tile
import concourse.mybir as mybir
from concourse import bacc
from concourse.masks import make_identity

B, S, D, H = 4, 2048, 768, 12
DK, P = 64, 128
HL = H // 2            # 6 heads per core
DL = HL * DK           # 384 local features
KD = D // P            # 6 contraction chunks over d
MT = DL // P           # 3 head-pairs (row-tiles of qT/kT/ctxT)
ST = S // P            # 16 s-tiles
W = 512                # attention i-block width
NIB = S // W           # 4 i-blocks
NEG = -1e9

CDT = mybir.dt.bfloat16
NP_CDT = ml_dtypes.bfloat16
F32 = mybir.dt.float32

N_CORES = 8


def _emit(nc, tc, xt_d, wq_d, wk_d, wv_d, wo_d, out_d):
    Exp = mybir.ActivationFunctionType.Exp
    Div = mybir.AluOpType.divide

    with tc.tile_pool(name="persist", bufs=1) as per, \
         tc.tile_pool(name="quad", bufs=1, space="PSUM") as qp, \
         tc.tile_pool(name="pc", bufs=2, space="PSUM") as cp, \
         tc.tile_pool(name="aux", bufs=1, space="PSUM") as ap, \
         tc.tile_pool(name="sb_e", bufs=3) as ep, \
         tc.tile_pool(name="sb_l", bufs=2) as lp, \
         tc.tile_pool(name="sb_o", bufs=2) as ob:
        xt = per.tile([P, KD, S], CDT)
        wq = per.tile([P, KD, DL], CDT)
        wk = per.tile([P, KD, DL], CDT)
        wv = per.tile([P, KD, DL], CDT)
        wo = per.tile([P, MT, D], CDT)
        qt = per.tile([P, MT, S], CDT)
        kt = per.tile([P, MT, S], CDT)
        v = per.tile([P, ST, HL, DK + 2], CDT)  # [v | ones | pad]
        ctxt = per.tile([P, MT, S], CDT)
        ident = per.tile([P, P], CDT)
        maskm = per.tile([P, P], CDT)

        # aux PSUM region: projection segments / out-projection tiles.
        aux = ap.tile([P, 2, 512], F32)

        # DMA priority: q/k path first (phase-0 projections), then wv,
        # wo last (needed only by the out-projection fillers).
        HS = S // 2
        for k in range(KD):
            nc.sync.dma_start(out=xt[:, k, 0:HS],
                              in_=xt_d[k * P:(k + 1) * P, 0:HS])
        for k in range(KD):
            nc.sync.dma_start(out=wq[:, k, :], in_=wq_d[k * P:(k + 1) * P, :])
            nc.sync.dma_start(out=wk[:, k, :], in_=wk_d[k * P:(k + 1) * P, :])
        for k in range(KD):
            nc.sync.dma_start(out=wv[:, k, :], in_=wv_d[k * P:(k + 1) * P, :])
        for k in range(KD):
            nc.sync.dma_start(out=xt[:, k, HS:S],
                              in_=xt_d[k * P:(k + 1) * P, HS:S])
        for m in range(MT):
            nc.sync.dma_start(out=wo[:, m, :], in_=wo_d[m * P:(m + 1) * P, :])

        nc.vector.memset(v[:, :, :, DK:], 0.0)
        nc.vector.memset(v[:, :, :, DK], 1.0)
        # Causal mask for diagonal tiles is injected on the PE itself:
        # matmul(lhsT=ident, rhs=maskm) writes maskm into the PSUM region
        # as the accumulation-group opener; the k.q matmul accumulates on
        # top. maskm[j, i] = NEG where i < j (invalid), 0 where i >= j.
        make_identity(nc, ident)
        nc.gpsimd.memset(maskm, 0.0)
        nc.gpsimd.affine_select(
            out=maskm, in_=maskm, compare_op=mybir.AluOpType.is_ge,
            fill=NEG, base=0, pattern=[[1, P]], channel_multiplier=-1)

        # HAM warmup: dummy matmuls keep the PE active window busy while
        # the input DMAs land. Output never read.
        for _ in range(40):
            nc.tensor.matmul(aux[:, 0, 0:P], lhsT=ident, rhs=maskm,
                             start=True, stop=True)

        # ---- projection emitters (also used as PE filler) ----
        def emit_qk_seg(wt, dst, mh, seg):
            slot = seg % 2
            s0 = seg * 512
            for k in range(KD):
                nc.tensor.matmul(
                    aux[:, slot, :], lhsT=wt[:, k, mh * P:(mh + 1) * P],
                    rhs=xt[:, k, s0:s0 + 512],
                    start=(k == 0), stop=(k == KD - 1))
            nc.vector.tensor_copy(dst[:, mh, s0:s0 + 512], aux[:, slot, :])

        def emit_v_st(st):
            slot = st % 2
            for k in range(KD):
                nc.tensor.matmul(
                    aux[:, slot, 0:DL], lhsT=xt[:, k, st * P:(st + 1) * P],
                    rhs=wv[:, k, :], start=(k == 0), stop=(k == KD - 1))
            nc.vector.tensor_copy(v[:, st, :, 0:DK], aux[:, slot, 0:DL])

        def emit_outproj_st(st):
            for n0, nn, slot in ((0, 512, 0), (512, 256, 1)):
                for m in range(MT):
                    nc.tensor.matmul(
                        aux[:, slot, 0:nn],
                        lhsT=ctxt[:, m, st * P:(st + 1) * P],
                        rhs=wo[:, m, n0:n0 + nn],
                        start=(m == 0), stop=(m == MT - 1))
            osb = ob.tile([P, D], CDT, tag="osb", name=f"osb_{st}")
            nc.vector.tensor_copy(osb[:, 0:512], aux[:, 0, :])
            nc.vector.tensor_copy(osb[:, 512:D], aux[:, 1, 0:D - 512])
            nc.sync.dma_start(out=out_d[st * P:(st + 1) * P, :], in_=osb)

        # ---- filler queue: (deadline t = mh*NIB+ib, closure) ----
        fillers = []

        def fill(n=1):
            for _ in range(n):
                if fillers:
                    fillers.pop(0)[1]()

        def flush(t):
            while fillers and fillers[0][0] <= t:
                fillers.pop(0)[1]()

        # ---- attention for one pair, one i-block ----
        def emit_att(mh, ib):
            heads = (2 * mh, 2 * mh + 1)
            i0 = ib * W
            njt = (i0 + W) // P
            quad = qp.tile([P, 4, W], F32, tag="quad", name=f"quad_{mh}_{ib}")
            pctxs = {h: cp.tile([P, W], F32, tag="pc",
                                name=f"pctx_{h}_{ib}") for h in heads}
            ets = {}
            for jt in range(njt + 2):
                # stage 1: scores + exp for jt
                if jt < njt:
                    sp = 2 * (jt % 2)
                    c0 = max(0, jt * P - i0)
                    diag = jt * P >= i0
                    for idx, h in enumerate(heads):
                        oh = idx * DK
                        slot = quad[:, sp + idx, :]
                        ktt = kt[oh:oh + DK, mh, jt * P:(jt + 1) * P]
                        if diag:
                            nc.tensor.matmul(slot[:, c0:c0 + P], lhsT=ident,
                                             rhs=maskm, start=True, stop=False)
                            nc.tensor.matmul(
                                slot[:, c0:c0 + P], lhsT=ktt,
                                rhs=qt[oh:oh + DK, mh, i0 + c0:i0 + c0 + P],
                                start=False, stop=True)
                            if c0 + P < W:
                                nc.tensor.matmul(
                                    slot[:, c0 + P:W], lhsT=ktt,
                                    rhs=qt[oh:oh + DK, mh, i0 + c0 + P:i0 + W],
                                    start=True, stop=True)
                        else:
                            nc.tensor.matmul(
                                slot, lhsT=ktt,
                                rhs=qt[oh:oh + DK, mh, i0:i0 + W],
                                start=True, stop=True)
                    et = ep.tile([P, 2, W], CDT, tag="et",
                                 name=f"et_{mh}_{ib}_{jt}")
                    nc.scalar.activation(et[:, :, c0:W],
                                         quad[:, sp:sp + 2, c0:W],
                                         Exp, scale=0.125)
                    ets[jt] = et
                # stage 2 (two jts behind): ctx for jt-2 -- scores(jt) and
                # scores(jt+1) both precede ctx(jt-1) in the frozen engine
                # order, so exp(jt)'s operands are always ready the moment
                # exp(jt-1) retires and ACT streams back-to-back
                if jt >= 2:
                    pj = jt - 2
                    pc0 = max(0, pj * P - i0)
                    for idx, h in enumerate(heads):
                        nc.tensor.matmul(
                            pctxs[h][0:DK + 2, pc0:W],
                            lhsT=v[:, pj, h, :], rhs=ets[pj][:, idx, pc0:W],
                            start=(pj == 0), stop=(pj == njt - 1))
                    del ets[pj]
                fill(1)
            # epilogue: one fast copy evacuates ctx rows + denom row to
            # SBUF (frees the pctx PSUM bank for the next i-block); the
            # normalize (recip+broadcast+mul) then runs off SBUF, fully
            # overlapped with the next block's attention.
            for idx, h in enumerate(heads):
                ev = lp.tile([DK, W], F32, tag="ev", name=f"ev_{h}_{ib}")
                nc.vector.tensor_copy(ev, pctxs[h][0:DK, :])
                lsb = lp.tile([1, W], F32, tag="lsb", name=f"lsb_{h}_{ib}")
                nc.vector.tensor_copy(lsb, pctxs[h][DK:DK + 1, :])
                rsb = lp.tile([1, W], F32, tag="rsb", name=f"rsb_{h}_{ib}")
                nc.vector.reciprocal_approx_fast(rsb, lsb)
                lb = lp.tile([DK, W], F32, tag="lb", name=f"lb_{h}_{ib}")
                nc.gpsimd.partition_broadcast(lb, rsb, channels=DK)
                nc.vector.tensor_mul(
                    ctxt[idx * DK:(idx + 1) * DK, mh, i0:i0 + W],
                    ev, lb)

        # ---- phase 0: minimum to start (0, ib0): q/k seg0 + v st0..3 ----
        emit_qk_seg(wq, qt, 0, 0)
        emit_qk_seg(wk, kt, 0, 0)
        for st in range(4):
            emit_v_st(st)

        # ---- fillers with deadlines (t = mh*NIB + ib) ----
        # qk(p, seg) needed by (p, ib=seg); v(st) by (0, ib=st//4);
        # out-projections are appended during pair 2.
        for p in range(MT):
            for seg in range(4):
                if p == 0 and seg == 0:
                    continue
                fillers.append((p * NIB + seg,
                                lambda p=p, seg=seg: emit_qk_seg(wq, qt, p, seg)))
                fillers.append((p * NIB + seg,
                                lambda p=p, seg=seg: emit_qk_seg(wk, kt, p, seg)))
        for st in range(4, ST):
            fillers.append((st // 4, lambda st=st: emit_v_st(st)))
        fillers.sort(key=lambda x: x[0])

        # ---- main loop: pair-outer, i-block inner ----
        for mh in range(MT):
            for ib in range(NIB):
                flush(mh * NIB + ib)
                emit_att(mh, ib)
                if mh == MT - 1:
                    # this i-block's ctxt is now complete for all pairs:
                    # queue its out-projection as filler for later blocks
                    for st in range(ib * (W // P), (ib + 1) * (W // P)):
                        fillers.append((99, lambda st=st: emit_outproj_st(st)))
        while fillers:
            fill(1)


def build_nc():
    nc = bacc.Bacc(trn_type="TRN2", target_bir_lowering=False, debug=False)
    xt_d = nc.dram_tensor("xt", [D, S], CDT, kind="ExternalInput").ap()
    wq_d = nc.dram_tensor("wq", [D, DL], CDT, kind="ExternalInput").ap()
    wk_d = nc.dram_tensor("wk", [D, DL], CDT, kind="ExternalInput").ap()
    wv_d = nc.dram_tensor("wv", [D, DL], CDT, kind="ExternalInput").ap()
    wo_d = nc.dram_tensor("wo", [DL, D], CDT, kind="ExternalInput").ap()
    out_d = nc.dram_tensor("out", [S, D], CDT, kind="ExternalOutput").ap()
    with tile.TileContext(nc) as tc:
        _emit(nc, tc, xt_d, wq_d, wk_d, wv_d, wo_d, out_d)
    nc.compile()
    return nc


def make_in_maps(x, Wq, Wk, Wv, Wo):
    in_maps = []
    for c in range(N_CORES):
        b, g = c // 2, c % 2
        hsl = slice(g * DL, (g + 1) * DL)
        in_maps.append({
            "xt": np.ascontiguousarray(x[b].T).astype(NP_CDT),
            "wq": np.ascontiguousarray(Wq[hsl, :].T).astype(NP_CDT),
            "wk": np.ascontiguousarray(Wk[hsl, :].T).astype(NP_CDT),
            "wv": np.ascontiguousarray(Wv[hsl, :].T).astype(NP_CDT),
            "wo": np.ascontiguousarray(Wo[:, hsl].T).astype(NP_CDT),
        })
    return in_maps


_BUILT = None
LAST_RESULT = None


def _install_ntff_hook():
    """Provide the antenv.axon_hooks module run_bass_kernel_spmd expects
    for NTFF profiling under axon (the agent image ships only a stub
    antenv package)."""
    import sys
    import types
    if "antenv.axon_hooks" in sys.modules:
        return
    mod = types.ModuleType("antenv.axon_hooks")
    mod._hook = None

    def set_axon_ntff_profile_hook(h):
        mod._hook = h

    def get_axon_ntff_profile_hook():
        return mod._hook

    mod.set_axon_ntff_profile_hook = set_axon_ntff_profile_hook
    mod.get_axon_ntff_profile_hook = get_axon_ntff_profile_hook
    sys.modules["antenv.axon_hooks"] = mod
    import antenv
    antenv.axon_hooks = mod
    try:
        from trn_agent_boot.trn_boot import _ntff_profile_via_ctypes
        hook = _ntff_profile_via_ctypes("/opt/axon/libaxon_pjrt.so")
        if hook is not None:
            mod._hook = hook
    except Exception:
        pass


def kernel(**inputs):
    global _BUILT, LAST_RESULT
    from concourse.bass_utils import run_bass_kernel_spmd

    x = np.asarray(inputs["x"], np.float32)
    Wq = np.asarray(inputs["Wq"], np.float32)
    Wk = np.asarray(inputs["Wk"], np.float32)
    Wv = np.asarray(inputs["Wv"], np.float32)
    Wo = np.asarray(inputs["Wo"], np.float32)
    bo = np.asarray(inputs["bo"], np.float32)

    if _BUILT is None:
        _BUILT = build_nc()
    nc = _BUILT

    trace = bool(int(os.environ.get("KTRACE", "0")))
    if trace:
        _install_ntff_hook()
    in_maps = make_in_maps(x, Wq, Wk, Wv, Wo)
    res = run_bass_kernel_spmd(
        nc, in_maps, core_ids=list(range(N_CORES)), trace=trace)
    LAST_RESULT = res

    out = np.empty((B, S, D), np.float32)
    for b in range(B):
        out[b] = (res.results[2 * b]["out"].astype(np.float32)
                  + res.results[2 * b + 1]["out"].astype(np.float32))
    out += bo
    return out


# revision 14
# speedup vs baseline: 1.4196x; 1.0526x over previous
"""Multi-head causal attention on 8 TRN2 NeuronCores.

Problem: B=4, S=2048, D=768, H=12 heads (dk=64), causal mask, f32.

Sharding: 8 cores = 4 batches x 2 head-groups (6 heads each).
Core c handles batch c//2 and heads [6*(c%2), 6*(c%2)+6).
Each core computes its partial output projection (over its 384 local
features) in bf16; the pair-sum (tensor-parallel all-reduce after Wo)
and the bo bias add happen at unshard time on the host in f32.

Design (v2), engine-balance driven:
 - ACT (ScalarE) exp is the hard floor: 1 elem/lane/cycle. The exp of
   both heads of a pair is issued as ONE instruction over a [128,2,W]
   PSUM view (adjacent quad slots) to halve the 352-cycle/instr
   overhead.
 - Scores matmuls are K=64 and run 2x via PE row tiling: head A on
   tile (0,0) (kt/qt partitions 0-63), head B on (64,0). Verified on
   HW: paired K=64 N=512 matmuls take 132ns vs 261ns serial. The A/B
   slot pair is freed by a single exp, so no pool-WAR semaphore lands
   between the paired matmuls.
 - Attention i-block width W=512. PSUM: quad [128,4,512] (4 banks,
   double-buffered slot pairs) + pctx A/B (2 banks) + aux [128,2,512]
   (2 banks, shared by projection fillers / out-projection).
 - All projection work (v, q/k of later pairs, out-projection of done
   i-blocks) is interleaved into the attention stream as PE filler so
   the PE never idles (keeps the HAM clock gate at full speed) while
   ACT streams exps.
 - Softmax denominator rides the ctx matmul as a ones-column (M=65);
   epilogue: gpsimd partition-broadcast of the denom row + one DVE
   divide straight into ctxt (bf16).
"""

import os
import numpy as np
import ml_dtypes

import concourse.bass as bass
import concourse.tile as tile
import concourse.mybir as mybir
from concourse import bacc
from concourse.masks import make_identity

B, S, D, H = 4, 2048, 768, 12
DK, P = 64, 128
HL = H // 2            # 6 heads per core
DL = HL * DK           # 384 local features
KD = D // P            # 6 contraction chunks over d
MT = DL // P           # 3 head-pairs (row-tiles of qT/kT/ctxT)
ST = S // P            # 16 s-tiles
W = 512                # attention i-block width
NIB = S // W           # 4 i-blocks
NEG = -1e9

CDT = mybir.dt.bfloat16
NP_CDT = ml_dtypes.bfloat16
F32 = mybir.dt.float32

N_CORES = 8


def _emit(nc, tc, xt_d, wq_d, wk_d, wv_d, wo_d, out_d):
    Exp = mybir.ActivationFunctionType.Exp
    Div = mybir.AluOpType.divide

    with tc.tile_pool(name="persist", bufs=1) as per, \
         tc.tile_pool(name="quad", bufs=1, space="PSUM") as qp, \
         tc.tile_pool(name="pc", bufs=2, space="PSUM") as cp, \
         tc.tile_pool(name="aux", bufs=1, space="PSUM") as ap, \
         tc.tile_pool(name="sb_e", bufs=3) as ep, \
         tc.tile_pool(name="sb_l", bufs=2) as lp, \
         tc.tile_pool(name="sb_o", bufs=2) as ob:
        xt = per.tile([P, KD, S], CDT)
        wq = per.tile([P, KD, DL], CDT)
        wk = per.tile([P, KD, DL], CDT)
        wv = per.tile([P, KD, DL], CDT)
        wo = per.tile([P, MT, D], CDT)
        qt = per.tile([P, MT, S], CDT)
        kt = per.tile([P, MT, S], CDT)
        v = per.tile([P, ST, HL, DK + 2], CDT)  # [v | ones | pad]
        ctxt = per.tile([P, MT, S], CDT)
        ident = per.tile([P, P], CDT)
        maskm = per.tile([P, P], CDT)

        # aux PSUM region: projection segments / out-projection tiles.
        aux = ap.tile([P, 2, 512], F32)

        # DMA priority: q/k path first (phase-0 projections), then wv,
        # wo last (needed only by the out-projection fillers).
        HS = S // 2
        for k in range(KD):
            nc.sync.dma_start(out=xt[:, k, 0:HS],
                              in_=xt_d[k * P:(k + 1) * P, 0:HS])
        for k in range(KD):
            nc.sync.dma_start(out=wq[:, k, :], in_=wq_d[k * P:(k + 1) * P, :])
            nc.sync.dma_start(out=wk[:, k, :], in_=wk_d[k * P:(k + 1) * P, :])
        for k in range(KD):
            nc.sync.dma_start(out=wv[:, k, :], in_=wv_d[k * P:(k + 1) * P, :])
        for k in range(KD):
            nc.sync.dma_start(out=xt[:, k, HS:S],
                              in_=xt_d[k * P:(k + 1) * P, HS:S])
        for m in range(MT):
            nc.sync.dma_start(out=wo[:, m, :], in_=wo_d[m * P:(m + 1) * P, :])

        nc.vector.memset(v[:, :, :, DK:], 0.0)
        nc.vector.memset(v[:, :, :, DK], 1.0)
        # Causal mask for diagonal tiles is injected on the PE itself:
        # matmul(lhsT=ident, rhs=maskm) writes maskm into the PSUM region
        # as the accumulation-group opener; the k.q matmul accumulates on
        # top. maskm[j, i] = NEG where i < j (invalid), 0 where i >= j.
        make_identity(nc, ident)
        nc.gpsimd.memset(maskm, 0.0)
        nc.gpsimd.affine_select(
            out=maskm, in_=maskm, compare_op=mybir.AluOpType.is_ge,
            fill=NEG, base=0, pattern=[[1, P]], channel_multiplier=-1)

        # HAM warmup: dummy matmuls keep the PE active window busy while
        # the input DMAs land. Output never read.
        for _ in range(40):
            nc.tensor.matmul(aux[:, 0, 0:P], lhsT=ident, rhs=maskm,
                             start=True, stop=True)

        # ---- projection emitters (also used as PE filler) ----
        # Each is split into two ~0.5us halves so the filler stream can
        # plug every per-jt PE bubble (a fully-busy PE is what keeps the
        # HAM clock gate at 2.4 GHz). Halves of one item are adjacent in
        # the FIFO, so the open aux accumulation is never interleaved.
        def emit_qk_seg(wt, dst, mh, seg, half):
            slot = seg % 2
            s0 = seg * 512
            ks = range(0, 3) if half == 0 else range(3, KD)
            for k in ks:
                nc.tensor.matmul(
                    aux[:, slot, :], lhsT=wt[:, k, mh * P:(mh + 1) * P],
                    rhs=xt[:, k, s0:s0 + 512],
                    start=(k == 0), stop=(k == KD - 1))
            if half == 1:
                nc.vector.tensor_copy(dst[:, mh, s0:s0 + 512], aux[:, slot, :])

        def emit_v_st(st, half):
            slot = st % 2
            ks = range(0, 3) if half == 0 else range(3, KD)
            for k in ks:
                nc.tensor.matmul(
                    aux[:, slot, 0:DL], lhsT=xt[:, k, st * P:(st + 1) * P],
                    rhs=wv[:, k, :], start=(k == 0), stop=(k == KD - 1))
            if half == 1:
                nc.vector.tensor_copy(v[:, st, :, 0:DK], aux[:, slot, 0:DL])

        def emit_outproj_st(st, half):
            n0, nn, slot = ((0, 512, 0), (512, 256, 1))[half]
            for m in range(MT):
                nc.tensor.matmul(
                    aux[:, slot, 0:nn],
                    lhsT=ctxt[:, m, st * P:(st + 1) * P],
                    rhs=wo[:, m, n0:n0 + nn],
                    start=(m == 0), stop=(m == MT - 1))
            if half == 1:
                osb = ob.tile([P, D], CDT, tag="osb", name=f"osb_{st}")
                nc.vector.tensor_copy(osb[:, 0:512], aux[:, 0, :])
                nc.vector.tensor_copy(osb[:, 512:D], aux[:, 1, 0:D - 512])
                nc.sync.dma_start(out=out_d[st * P:(st + 1) * P, :], in_=osb)

        # ---- filler queue: (deadline t = mh*NIB+ib, closure) ----
        fillers = []

        def fill(n=1):
            for _ in range(n):
                if fillers:
                    fillers.pop(0)[1]()

        def flush(t):
            while fillers and fillers[0][0] <= t:
                fillers.pop(0)[1]()

        # ---- attention for one pair, one i-block ----
        def emit_att(mh, ib):
            heads = (2 * mh, 2 * mh + 1)
            i0 = ib * W
            njt = (i0 + W) // P
            quad = qp.tile([P, 4, W], F32, tag="quad", name=f"quad_{mh}_{ib}")
            pctxs = {h: cp.tile([P, W], F32, tag="pc",
                                name=f"pctx_{h}_{ib}") for h in heads}
            ets = {}
            for jt in range(njt + 2):
                # stage 1: scores + exp for jt
                if jt < njt:
                    sp = 2 * (jt % 2)
                    c0 = max(0, jt * P - i0)
                    diag = jt * P >= i0
                    for idx, h in enumerate(heads):
                        oh = idx * DK
                        slot = quad[:, sp + idx, :]
                        ktt = kt[oh:oh + DK, mh, jt * P:(jt + 1) * P]
                        if diag:
                            nc.tensor.matmul(slot[:, c0:c0 + P], lhsT=ident,
                                             rhs=maskm, start=True, stop=False)
                            nc.tensor.matmul(
                                slot[:, c0:c0 + P], lhsT=ktt,
                                rhs=qt[oh:oh + DK, mh, i0 + c0:i0 + c0 + P],
                                start=False, stop=True)
                            if c0 + P < W:
                                nc.tensor.matmul(
                                    slot[:, c0 + P:W], lhsT=ktt,
                                    rhs=qt[oh:oh + DK, mh, i0 + c0 + P:i0 + W],
                                    start=True, stop=True)
                        else:
                            nc.tensor.matmul(
                                slot, lhsT=ktt,
                                rhs=qt[oh:oh + DK, mh, i0:i0 + W],
                                start=True, stop=True)
                    et = ep.tile([P, 2, W], CDT, tag="et",
                                 name=f"et_{mh}_{ib}_{jt}")
                    nc.scalar.activation(et[:, :, c0:W],
                                         quad[:, sp:sp + 2, c0:W],
                                         Exp, scale=0.125)
                    ets[jt] = et
                # stage 2 (two jts behind): ctx for jt-2 -- scores(jt) and
                # scores(jt+1) both precede ctx(jt-1) in the frozen engine
                # order, so exp(jt)'s operands are always ready the moment
                # exp(jt-1) retires and ACT streams back-to-back
                if jt >= 2:
                    pj = jt - 2
                    pc0 = max(0, pj * P - i0)
                    for idx, h in enumerate(heads):
                        nc.tensor.matmul(
                            pctxs[h][0:DK + 2, pc0:W],
                            lhsT=v[:, pj, h, :], rhs=ets[pj][:, idx, pc0:W],
                            start=(pj == 0), stop=(pj == njt - 1))
                    del ets[pj]
                fill(1)
            # epilogue: one fast copy evacuates ctx rows + denom row to
            # SBUF (frees the pctx PSUM bank for the next i-block); the
            # normalize (recip+broadcast+mul) then runs off SBUF, fully
            # overlapped with the next block's attention.
            for idx, h in enumerate(heads):
                ev = lp.tile([DK, W], F32, tag="ev", name=f"ev_{h}_{ib}")
                nc.vector.tensor_copy(ev, pctxs[h][0:DK, :])
                lsb = lp.tile([1, W], F32, tag="lsb", name=f"lsb_{h}_{ib}")
                nc.vector.tensor_copy(lsb, pctxs[h][DK:DK + 1, :])
                rsb = lp.tile([1, W], F32, tag="rsb", name=f"rsb_{h}_{ib}")
                nc.vector.reciprocal_approx_fast(rsb, lsb)
                lb = lp.tile([DK, W], F32, tag="lb", name=f"lb_{h}_{ib}")
                nc.gpsimd.partition_broadcast(lb, rsb, channels=DK)
                nc.vector.tensor_mul(
                    ctxt[idx * DK:(idx + 1) * DK, mh, i0:i0 + W],
                    ev, lb)

        # ---- phase 0: minimum to start (0, ib0): q/k seg0 + v st0..3 ----
        for half in (0, 1):
            emit_qk_seg(wq, qt, 0, 0, half)
        for half in (0, 1):
            emit_qk_seg(wk, kt, 0, 0, half)
        for st in range(4):
            for half in (0, 1):
                emit_v_st(st, half)

        # ---- fillers with deadlines (t = mh*NIB + ib) ----
        # qk(p, seg) needed by (p, ib=seg); v(st) by (0, ib=st//4);
        # out-projections are appended during pair 2.
        for p in range(MT):
            for seg in range(4):
                if p == 0 and seg == 0:
                    continue
                for wt, dst in ((wq, qt), (wk, kt)):
                    for half in (0, 1):
                        fillers.append(
                            (p * NIB + seg,
                             lambda wt=wt, dst=dst, p=p, seg=seg, half=half:
                             emit_qk_seg(wt, dst, p, seg, half)))
        for st in range(4, ST):
            for half in (0, 1):
                fillers.append((st // 4,
                                lambda st=st, half=half: emit_v_st(st, half)))
        fillers.sort(key=lambda x: x[0])

        # ---- main loop: pair-outer, i-block inner ----
        for mh in range(MT):
            for ib in range(NIB):
                flush(mh * NIB + ib)
                emit_att(mh, ib)
                if mh == MT - 1:
                    # this i-block's ctxt is now complete for all pairs:
                    # queue its out-projection as filler for later blocks
                    for st in range(ib * (W // P), (ib + 1) * (W // P)):
                        for half in (0, 1):
                            fillers.append(
                                (99, lambda st=st, half=half:
                                 emit_outproj_st(st, half)))
        while fillers:
            fill(1)


def build_nc():
    nc = bacc.Bacc(trn_type="TRN2", target_bir_lowering=False, debug=False)
    xt_d = nc.dram_tensor("xt", [D, S], CDT, kind="ExternalInput").ap()
    wq_d = nc.dram_tensor("wq", [D, DL], CDT, kind="ExternalInput").ap()
    wk_d = nc.dram_tensor("wk", [D, DL], CDT, kind="ExternalInput").ap()
    wv_d = nc.dram_tensor("wv", [D, DL], CDT, kind="ExternalInput").ap()
    wo_d = nc.dram_tensor("wo", [DL, D], CDT, kind="ExternalInput").ap()
    out_d = nc.dram_tensor("out", [S, D], CDT, kind="ExternalOutput").ap()
    with tile.TileContext(nc) as tc:
        _emit(nc, tc, xt_d, wq_d, wk_d, wv_d, wo_d, out_d)
    nc.compile()
    return nc


def make_in_maps(x, Wq, Wk, Wv, Wo):
    in_maps = []
    for c in range(N_CORES):
        b, g = c // 2, c % 2
        hsl = slice(g * DL, (g + 1) * DL)
        in_maps.append({
            "xt": np.ascontiguousarray(x[b].T).astype(NP_CDT),
            "wq": np.ascontiguousarray(Wq[hsl, :].T).astype(NP_CDT),
            "wk": np.ascontiguousarray(Wk[hsl, :].T).astype(NP_CDT),
            "wv": np.ascontiguousarray(Wv[hsl, :].T).astype(NP_CDT),
            "wo": np.ascontiguousarray(Wo[:, hsl].T).astype(NP_CDT),
        })
    return in_maps


_BUILT = None
LAST_RESULT = None


def _install_ntff_hook():
    """Provide the antenv.axon_hooks module run_bass_kernel_spmd expects
    for NTFF profiling under axon (the agent image ships only a stub
    antenv package)."""
    import sys
    import types
    if "antenv.axon_hooks" in sys.modules:
        return
    mod = types.ModuleType("antenv.axon_hooks")
    mod._hook = None

    def set_axon_ntff_profile_hook(h):
        mod._hook = h

    def get_axon_ntff_profile_hook():
        return mod._hook

    mod.set_axon_ntff_profile_hook = set_axon_ntff_profile_hook
    mod.get_axon_ntff_profile_hook = get_axon_ntff_profile_hook
    sys.modules["antenv.axon_hooks"] = mod
    import antenv
    antenv.axon_hooks = mod
    try:
        from trn_agent_boot.trn_boot import _ntff_profile_via_ctypes
        hook = _ntff_profile_via_ctypes("/opt/axon/libaxon_pjrt.so")
        if hook is not None:
            mod._hook = hook
    except Exception:
        pass


def kernel(**inputs):
    global _BUILT, LAST_RESULT
    from concourse.bass_utils import run_bass_kernel_spmd

    x = np.asarray(inputs["x"], np.float32)
    Wq = np.asarray(inputs["Wq"], np.float32)
    Wk = np.asarray(inputs["Wk"], np.float32)
    Wv = np.asarray(inputs["Wv"], np.float32)
    Wo = np.asarray(inputs["Wo"], np.float32)
    bo = np.asarray(inputs["bo"], np.float32)

    if _BUILT is None:
        _BUILT = build_nc()
    nc = _BUILT

    trace = bool(int(os.environ.get("KTRACE", "0")))
    if trace:
        _install_ntff_hook()
    in_maps = make_in_maps(x, Wq, Wk, Wv, Wo)
    res = run_bass_kernel_spmd(
        nc, in_maps, core_ids=list(range(N_CORES)), trace=trace)
    LAST_RESULT = res

    out = np.empty((B, S, D), np.float32)
    for b in range(B):
        out[b] = (res.results[2 * b]["out"].astype(np.float32)
                  + res.results[2 * b + 1]["out"].astype(np.float32))
    out += bo
    return out


# revision 15
# speedup vs baseline: 1.4614x; 1.0295x over previous
"""Multi-head causal attention on 8 TRN2 NeuronCores.

Problem: B=4, S=2048, D=768, H=12 heads (dk=64), causal mask, f32.

Sharding: 8 cores = 4 batches x 2 head-groups (6 heads each).
Core c handles batch c//2 and heads [6*(c%2), 6*(c%2)+6).
Each core computes its partial output projection (over its 384 local
features) in bf16; the pair-sum (tensor-parallel all-reduce after Wo)
and the bo bias add happen at unshard time on the host in f32.

Design (v2), engine-balance driven:
 - ACT (ScalarE) exp is the hard floor: 1 elem/lane/cycle. The exp of
   both heads of a pair is issued as ONE instruction over a [128,2,W]
   PSUM view (adjacent quad slots) to halve the 352-cycle/instr
   overhead.
 - Scores matmuls are K=64 and run 2x via PE row tiling: head A on
   tile (0,0) (kt/qt partitions 0-63), head B on (64,0). Verified on
   HW: paired K=64 N=512 matmuls take 132ns vs 261ns serial. The A/B
   slot pair is freed by a single exp, so no pool-WAR semaphore lands
   between the paired matmuls.
 - Attention i-block width W=512. PSUM: quad [128,4,512] (4 banks,
   double-buffered slot pairs) + pctx A/B (2 banks) + aux [128,2,512]
   (2 banks, shared by projection fillers / out-projection).
 - All projection work (v, q/k of later pairs, out-projection of done
   i-blocks) is interleaved into the attention stream as PE filler so
   the PE never idles (keeps the HAM clock gate at full speed) while
   ACT streams exps.
 - Softmax denominator rides the ctx matmul as a ones-column (M=65);
   epilogue: gpsimd partition-broadcast of the denom row + one DVE
   divide straight into ctxt (bf16).
"""

import os
import numpy as np
import ml_dtypes

import concourse.bass as bass
import concourse.tile as tile
import concourse.mybir as mybir
from concourse import bacc
from concourse.masks import make_identity

B, S, D, H = 4, 2048, 768, 12
DK, P = 64, 128
HL = H // 2            # 6 heads per core
DL = HL * DK           # 384 local features
KD = D // P            # 6 contraction chunks over d
MT = DL // P           # 3 head-pairs (row-tiles of qT/kT/ctxT)
ST = S // P            # 16 s-tiles
W = 512                # attention i-block width
NIB = S // W           # 4 i-blocks
NEG = -1e9

CDT = mybir.dt.bfloat16
NP_CDT = ml_dtypes.bfloat16
F32 = mybir.dt.float32

N_CORES = 8


def _emit(nc, tc, xt_d, wq_d, wk_d, wv_d, wo_d, out_d):
    Exp = mybir.ActivationFunctionType.Exp
    Div = mybir.AluOpType.divide

    with tc.tile_pool(name="persist", bufs=1) as per, \
         tc.tile_pool(name="quad", bufs=1, space="PSUM") as qp, \
         tc.tile_pool(name="pc", bufs=2, space="PSUM") as cp, \
         tc.tile_pool(name="aux", bufs=1, space="PSUM") as ap, \
         tc.tile_pool(name="sb_e", bufs=3) as ep, \
         tc.tile_pool(name="sb_l", bufs=2) as lp, \
         tc.tile_pool(name="sb_o", bufs=2) as ob:
        xt = per.tile([P, KD, S], CDT)
        wq = per.tile([P, KD, DL], CDT)
        wk = per.tile([P, KD, DL], CDT)
        wv = per.tile([P, KD, DL], CDT)
        wo = per.tile([P, MT, D], CDT)
        qt = per.tile([P, MT, S], CDT)
        kt = per.tile([P, MT, S], CDT)
        v = per.tile([P, ST, HL, DK + 2], CDT)  # [v | ones | pad]
        ctxt = per.tile([P, MT, S], CDT)
        ident = per.tile([P, P], CDT)
        maskm = per.tile([P, P], CDT)

        # aux PSUM region: projection segments / out-projection tiles.
        aux = ap.tile([P, 2, 512], F32)

        # DMA priority: q/k path first (phase-0 projections), then wv,
        # wo last (needed only by the out-projection fillers).
        HS = S // 2
        for k in range(KD):
            nc.sync.dma_start(out=xt[:, k, 0:HS],
                              in_=xt_d[k * P:(k + 1) * P, 0:HS])
        for k in range(KD):
            nc.sync.dma_start(out=wq[:, k, :], in_=wq_d[k * P:(k + 1) * P, :])
            nc.sync.dma_start(out=wk[:, k, :], in_=wk_d[k * P:(k + 1) * P, :])
        for k in range(KD):
            nc.sync.dma_start(out=wv[:, k, :], in_=wv_d[k * P:(k + 1) * P, :])
        for k in range(KD):
            nc.sync.dma_start(out=xt[:, k, HS:S],
                              in_=xt_d[k * P:(k + 1) * P, HS:S])
        for m in range(MT):
            nc.sync.dma_start(out=wo[:, m, :], in_=wo_d[m * P:(m + 1) * P, :])

        nc.vector.memset(v[:, :, :, DK:], 0.0)
        nc.vector.memset(v[:, :, :, DK], 1.0)
        # Causal mask for diagonal tiles is injected on the PE itself:
        # matmul(lhsT=ident, rhs=maskm) writes maskm into the PSUM region
        # as the accumulation-group opener; the k.q matmul accumulates on
        # top. maskm[j, i] = NEG where i < j (invalid), 0 where i >= j.
        make_identity(nc, ident)
        nc.gpsimd.memset(maskm, 0.0)
        nc.gpsimd.affine_select(
            out=maskm, in_=maskm, compare_op=mybir.AluOpType.is_ge,
            fill=NEG, base=0, pattern=[[1, P]], channel_multiplier=-1)

        # HAM warmup: dummy matmuls keep the PE active window busy while
        # the input DMAs land. Output never read.
        for _ in range(40):
            nc.tensor.matmul(aux[:, 0, 0:P], lhsT=ident, rhs=maskm,
                             start=True, stop=True)

        # ---- projection emitters (also used as PE filler) ----
        # Each is split into two ~0.5us halves so the filler stream can
        # plug every per-jt PE bubble (a fully-busy PE is what keeps the
        # HAM clock gate at 2.4 GHz). Halves of one item are adjacent in
        # the FIFO, so the open aux accumulation is never interleaved.
        def emit_qk_seg(wt, dst, mh, seg, part):
            slot = seg % 2
            s0 = seg * 512
            for k in (2 * part, 2 * part + 1):
                nc.tensor.matmul(
                    aux[:, slot, :], lhsT=wt[:, k, mh * P:(mh + 1) * P],
                    rhs=xt[:, k, s0:s0 + 512],
                    start=(k == 0), stop=(k == KD - 1))
            if part == 2:
                nc.vector.tensor_copy(dst[:, mh, s0:s0 + 512], aux[:, slot, :])

        def emit_v_st(st, part):
            slot = st % 2
            for k in (2 * part, 2 * part + 1):
                nc.tensor.matmul(
                    aux[:, slot, 0:DL], lhsT=xt[:, k, st * P:(st + 1) * P],
                    rhs=wv[:, k, :], start=(k == 0), stop=(k == KD - 1))
            if part == 2:
                nc.vector.tensor_copy(v[:, st, :, 0:DK], aux[:, slot, 0:DL])

        # out-projection split into 3 two-matmul parts across the two
        # aux slots: (s0:m0 m1), (s0:m2, s1:m0), (s1:m1 m2 + evacuate)
        def emit_outproj_st(st, part):
            steps = (((0, 0), (0, 1)), ((0, 2), (1, 0)),
                     ((1, 1), (1, 2)))[part]
            for slot, m in steps:
                n0, nn = (0, 512) if slot == 0 else (512, 256)
                nc.tensor.matmul(
                    aux[:, slot, 0:nn],
                    lhsT=ctxt[:, m, st * P:(st + 1) * P],
                    rhs=wo[:, m, n0:n0 + nn],
                    start=(m == 0), stop=(m == MT - 1))
            if part == 2:
                osb = ob.tile([P, D], CDT, tag="osb", name=f"osb_{st}")
                nc.vector.tensor_copy(osb[:, 0:512], aux[:, 0, :])
                nc.vector.tensor_copy(osb[:, 512:D], aux[:, 1, 0:D - 512])
                nc.sync.dma_start(out=out_d[st * P:(st + 1) * P, :], in_=osb)

        # ---- filler queue: (deadline t = mh*NIB+ib, closure) ----
        fillers = []

        def fill(n=1):
            for _ in range(n):
                if fillers:
                    fillers.pop(0)[1]()

        def flush(t):
            while fillers and fillers[0][0] <= t:
                fillers.pop(0)[1]()

        # ---- attention for one pair, one i-block ----
        def emit_att(mh, ib):
            heads = (2 * mh, 2 * mh + 1)
            i0 = ib * W
            njt = (i0 + W) // P
            quad = qp.tile([P, 4, W], F32, tag="quad", name=f"quad_{mh}_{ib}")
            pctxs = {h: cp.tile([P, W], F32, tag="pc",
                                name=f"pctx_{h}_{ib}") for h in heads}
            ets = {}
            for jt in range(njt + 2):
                # stage 1: scores + exp for jt
                if jt < njt:
                    sp = 2 * (jt % 2)
                    c0 = max(0, jt * P - i0)
                    diag = jt * P >= i0
                    for idx, h in enumerate(heads):
                        oh = idx * DK
                        slot = quad[:, sp + idx, :]
                        ktt = kt[oh:oh + DK, mh, jt * P:(jt + 1) * P]
                        if diag:
                            nc.tensor.matmul(slot[:, c0:c0 + P], lhsT=ident,
                                             rhs=maskm, start=True, stop=False)
                            nc.tensor.matmul(
                                slot[:, c0:c0 + P], lhsT=ktt,
                                rhs=qt[oh:oh + DK, mh, i0 + c0:i0 + c0 + P],
                                start=False, stop=True)
                            if c0 + P < W:
                                nc.tensor.matmul(
                                    slot[:, c0 + P:W], lhsT=ktt,
                                    rhs=qt[oh:oh + DK, mh, i0 + c0 + P:i0 + W],
                                    start=True, stop=True)
                        else:
                            nc.tensor.matmul(
                                slot, lhsT=ktt,
                                rhs=qt[oh:oh + DK, mh, i0:i0 + W],
                                start=True, stop=True)
                    et = ep.tile([P, 2, W], CDT, tag="et",
                                 name=f"et_{mh}_{ib}_{jt}")
                    nc.scalar.activation(et[:, :, c0:W],
                                         quad[:, sp:sp + 2, c0:W],
                                         Exp, scale=0.125)
                    ets[jt] = et
                # stage 2 (two jts behind): ctx for jt-2 -- scores(jt) and
                # scores(jt+1) both precede ctx(jt-1) in the frozen engine
                # order, so exp(jt)'s operands are always ready the moment
                # exp(jt-1) retires and ACT streams back-to-back
                if jt >= 2:
                    pj = jt - 2
                    pc0 = max(0, pj * P - i0)
                    for idx, h in enumerate(heads):
                        nc.tensor.matmul(
                            pctxs[h][0:DK + 2, pc0:W],
                            lhsT=v[:, pj, h, :], rhs=ets[pj][:, idx, pc0:W],
                            start=(pj == 0), stop=(pj == njt - 1))
                    del ets[pj]
                fill(1)
            # epilogue: one fast copy evacuates ctx rows + denom row to
            # SBUF (frees the pctx PSUM bank for the next i-block); the
            # normalize (recip+broadcast+mul) then runs off SBUF, fully
            # overlapped with the next block's attention.
            for idx, h in enumerate(heads):
                ev = lp.tile([DK, W], F32, tag="ev", name=f"ev_{h}_{ib}")
                nc.vector.tensor_copy(ev, pctxs[h][0:DK, :])
                lsb = lp.tile([1, W], F32, tag="lsb", name=f"lsb_{h}_{ib}")
                nc.vector.tensor_copy(lsb, pctxs[h][DK:DK + 1, :])
                rsb = lp.tile([1, W], F32, tag="rsb", name=f"rsb_{h}_{ib}")
                nc.vector.reciprocal_approx_fast(rsb, lsb)
                lb = lp.tile([DK, W], F32, tag="lb", name=f"lb_{h}_{ib}")
                nc.gpsimd.partition_broadcast(lb, rsb, channels=DK)
                nc.vector.tensor_mul(
                    ctxt[idx * DK:(idx + 1) * DK, mh, i0:i0 + W],
                    ev, lb)

        # ---- phase 0: minimum to start (0, ib0): q/k seg0 + v st0..3 ----
        for part in (0, 1, 2):
            emit_qk_seg(wq, qt, 0, 0, part)
        for part in (0, 1, 2):
            emit_qk_seg(wk, kt, 0, 0, part)
        for st in range(4):
            for part in (0, 1, 2):
                emit_v_st(st, part)

        # ---- fillers with deadlines (t = mh*NIB + ib) ----
        # qk(p, seg) needed by (p, ib=seg); v(st) by (0, ib=st//4);
        # out-projections are appended during pair 2.
        for p in range(MT):
            for seg in range(4):
                if p == 0 and seg == 0:
                    continue
                for wt, dst in ((wq, qt), (wk, kt)):
                    for part in (0, 1, 2):
                        fillers.append(
                            (p * NIB + seg,
                             lambda wt=wt, dst=dst, p=p, seg=seg, part=part:
                             emit_qk_seg(wt, dst, p, seg, part)))
        for st in range(4, ST):
            for part in (0, 1, 2):
                fillers.append((st // 4,
                                lambda st=st, part=part: emit_v_st(st, part)))
        fillers.sort(key=lambda x: x[0])

        # ---- main loop: pair-outer, i-block inner ----
        for mh in range(MT):
            for ib in range(NIB):
                flush(mh * NIB + ib)
                emit_att(mh, ib)
                if mh == MT - 1:
                    # this i-block's ctxt is now complete for all pairs:
                    # queue its out-projection as filler for later blocks
                    for st in range(ib * (W // P), (ib + 1) * (W // P)):
                        for part in (0, 1, 2):
                            fillers.append(
                                (99, lambda st=st, part=part:
                                 emit_outproj_st(st, part)))
        while fillers:
            fill(1)


def build_nc():
    nc = bacc.Bacc(trn_type="TRN2", target_bir_lowering=False, debug=False)
    xt_d = nc.dram_tensor("xt", [D, S], CDT, kind="ExternalInput").ap()
    wq_d = nc.dram_tensor("wq", [D, DL], CDT, kind="ExternalInput").ap()
    wk_d = nc.dram_tensor("wk", [D, DL], CDT, kind="ExternalInput").ap()
    wv_d = nc.dram_tensor("wv", [D, DL], CDT, kind="ExternalInput").ap()
    wo_d = nc.dram_tensor("wo", [DL, D], CDT, kind="ExternalInput").ap()
    out_d = nc.dram_tensor("out", [S, D], CDT, kind="ExternalOutput").ap()
    with tile.TileContext(nc) as tc:
        _emit(nc, tc, xt_d, wq_d, wk_d, wv_d, wo_d, out_d)
    nc.compile()
    return nc


def make_in_maps(x, Wq, Wk, Wv, Wo):
    in_maps = []
    for c in range(N_CORES):
        b, g = c // 2, c % 2
        hsl = slice(g * DL, (g + 1) * DL)
        in_maps.append({
            "xt": np.ascontiguousarray(x[b].T).astype(NP_CDT),
            "wq": np.ascontiguousarray(Wq[hsl, :].T).astype(NP_CDT),
            "wk": np.ascontiguousarray(Wk[hsl, :].T).astype(NP_CDT),
            "wv": np.ascontiguousarray(Wv[hsl, :].T).astype(NP_CDT),
            "wo": np.ascontiguousarray(Wo[:, hsl].T).astype(NP_CDT),
        })
    return in_maps


_BUILT = None
LAST_RESULT = None


def _install_ntff_hook():
    """Provide the antenv.axon_hooks module run_bass_kernel_spmd expects
    for NTFF profiling under axon (the agent image ships only a stub
    antenv package)."""
    import sys
    import types
    if "antenv.axon_hooks" in sys.modules:
        return
    mod = types.ModuleType("antenv.axon_hooks")
    mod._hook = None

    def set_axon_ntff_profile_hook(h):
        mod._hook = h

    def get_axon_ntff_profile_hook():
        return mod._hook

    mod.set_axon_ntff_profile_hook = set_axon_ntff_profile_hook
    mod.get_axon_ntff_profile_hook = get_axon_ntff_profile_hook
    sys.modules["antenv.axon_hooks"] = mod
    import antenv
    antenv.axon_hooks = mod
    try:
        from trn_agent_boot.trn_boot import _ntff_profile_via_ctypes
        hook = _ntff_profile_via_ctypes("/opt/axon/libaxon_pjrt.so")
        if hook is not None:
            mod._hook = hook
    except Exception:
        pass


def kernel(**inputs):
    global _BUILT, LAST_RESULT
    from concourse.bass_utils import run_bass_kernel_spmd

    x = np.asarray(inputs["x"], np.float32)
    Wq = np.asarray(inputs["Wq"], np.float32)
    Wk = np.asarray(inputs["Wk"], np.float32)
    Wv = np.asarray(inputs["Wv"], np.float32)
    Wo = np.asarray(inputs["Wo"], np.float32)
    bo = np.asarray(inputs["bo"], np.float32)

    if _BUILT is None:
        _BUILT = build_nc()
    nc = _BUILT

    trace = bool(int(os.environ.get("KTRACE", "0")))
    if trace:
        _install_ntff_hook()
    in_maps = make_in_maps(x, Wq, Wk, Wv, Wo)
    res = run_bass_kernel_spmd(
        nc, in_maps, core_ids=list(range(N_CORES)), trace=trace)
    LAST_RESULT = res

    out = np.empty((B, S, D), np.float32)
    for b in range(B):
        out[b] = (res.results[2 * b]["out"].astype(np.float32)
                  + res.results[2 * b + 1]["out"].astype(np.float32))
    out += bo
    return out
